# revision 24
# baseline (speedup 1.0000x reference)
"""Trainium2 Bass kernel for CSR sparse retrieval (scatter-add + top-k).

Strategy (per the doc-id sharding hint):
  * Host: gather the Q query posting lists (slices of rindices/cvalues given
    by ccol[indices]), scale them by the query weights, then shard the
    (doc, contrib) entries by document id across the 8 cores (doc-range
    split + doc sort inside each shard — the "split rindices/cvalues
    row-space by doc id" step).
  * Device (per core): segment-sum runs of equal doc ids (duplicates are
    adjacent after the doc sort; run lengths are tiny), keep the full sum
    only on each run's leader, and emit the per-partition top-8
    (values + indices) with VectorE max/max_index.
  * Host: reduce the 8 partial top-k candidate lists (plus the implicit
    zero-score untouched docs) to the exact global top-k with jax's
    tie-breaking order. A per-partition bound check (8th-best reported vs
    the final k-th value) proves exactness of the top-8-per-partition
    truncation; on the astronomically rare failure we recompute on host.
"""

import numpy as np

import concourse.bass as bass
import concourse.mybir as mybir
from concourse.bass_utils import run_bass_kernel_spmd

N_CORES = 8
P = 128            # SBUF partitions
HALO = 32          # max supported run length of equal doc ids
NEG_INF = -3.0e38  # suppression value for non-leader entries


def _slim_program(nc):
    """Post-build surgery on framework overhead around the kernel body.

    * Drop the four const-AP init memsets (f32 0/1, bf16 1, u8 127) on
      Pool: this kernel never reads those scratch constants, and Pool is
      the preamble's critical path.
    * Drop PE's and Activation's zero/bcreg RegisterMoves: those engines
      execute nothing here but barrier/drain sequencer ops.
    * Hoist the input DMA dispatch to the very start of SP's stream: the
      program has no semaphore clears (nothing to order against) and the
      DMA reads no registers, so its ~2.4us dispatch/descriptor-gen/
      transfer/sem pipeline overlaps the whole preamble.
    * Drop the Block-exit drains/barrier: every cross-engine ordering is
      carried by explicit semaphores (Pool exits only after the output-DMA
      completion sem), and the runtime already waits for each engine's
      halt, so the extra all-engine rendezvous only adds tail latency.
    """
    blocks = nc.m.functions[0].blocks
    b0 = blocks[0]
    idle = {mybir.EngineType.PE, mybir.EngineType.Activation}

    def dead(ins):
        nm = type(ins).__name__
        if nm == "InstMemset":
            return (getattr(ins.outs[0], "memref", "") or "").startswith("const-")
        if nm == "InstRegisterMove":
            return ins.engine in idle
        return False

    b0.instructions[:] = [ins for ins in b0.instructions if not dead(ins)]

    # hoist the SP input-DMA dispatch to the front of SP's stream
    # (transactional: locate both sites before mutating either)
    dma_site = None
    for b in blocks[1:]:
        for ins in b.instructions:
            if (type(ins).__name__ == "InstDMACopy"
                    and ins.engine == mybir.EngineType.SP):
                dma_site = (b, ins)
                break
        if dma_site is not None:
            break
    call_idx = next((i for i, ins in enumerate(b0.instructions)
                     if type(ins).__name__ == "InstCall"), None)
    assert dma_site is not None and call_idx is not None
    b, dma = dma_site
    b.instructions[:] = [i for i in b.instructions if i is not dma]
    out = list(b0.instructions)
    out.insert(call_idx + 1, dma)
    b0.instructions[:] = out

    # drop the Block-exit drains + all-engine barrier
    last = blocks[-1]
    last.instructions[:] = [
        i for i in last.instructions
        if type(i).__name__ not in ("InstEventSemaphore", "InstDrain")
    ]

    # engines consume only their own instruction stream, so the per-engine
    # block-chaining branches are structural no-ops; the tail branch after
    # the final semaphore wait otherwise delays each engine's halt
    for b in blocks:
        b.instructions[:] = [
            i for i in b.instructions
            if type(i).__name__ != "InstUnconditionalBranch"
        ]


def _c0_f32(T: int) -> int:
    """First f32 column of the contrib region in the packed int16 row."""
    return (2 * T + 3) // 4 + 1


def _xf(T: int) -> int:
    """f32 row width of the packed tile (>= 512B rows for full-rate DMA)."""
    return max(128, _c0_f32(T) + T)


def _build_bass(T: int, W: int, R: int):
    """Device program: one packed int16 [128, 2*XF] tile -> top-8/partition.

    Packed input per partition row (XF f32 columns = 2*XF int16):
      int16 cols 0..T-1  : local doc-run ranks (equal rank <=> same doc run)
      f32 cols c0..c0+T-1: contributions (cvalues * query weight)
    Window layout per partition row p (flat shard order, windows of W):
      cols 0..R-1  : pre-halo (preceding entries; a run ending in the
                     scored region starts inside the window since R >= the
                     max run length)
      cols R..R+W-1: this partition's W entries (scored)
      cols R+W..   : one successor doc col (run-end detection) + margin
    The segment sums come from ONE segmented scan:
      state[t] = eq[t-1] * state[t-1] + contrib[t]
    so a run's full sum lands on its LAST entry; entries whose successor
    continues the run are suppressed. int16 ranks + fp16 flags put the
    equality op in the DVE 2x perf mode.

    Output DRAM [128, 64] f32 (256B rows for the SWDGE scatter): cols 0:8 =
    top-8 values (descending), cols 8:16 = window indices (uint32 bits).
    """
    assert T >= W + R + 1, (T, W, R)
    c0 = _c0_f32(T)
    XF = _xf(T)
    nc = bass.Bass()
    pack_in = nc.dram_tensor("pack", [P, 2 * XF], mybir.dt.int16,
                             kind="ExternalInput")
    # 64-col (256B) rows: SWDGE scatter descriptors need a 256B-multiple
    # DRAM row stride. Host reads cols 0:16; the rest is scratch.
    out_pk = nc.dram_tensor("out", [P, 64], mybir.dt.float32,
                            kind="ExternalOutput")

    with (
        nc.sbuf_tensor([P, 2 * XF], mybir.dt.int16) as pack,
        nc.sbuf_tensor([P, T - 1], mybir.dt.float16) as eq,
        nc.sbuf_tensor([P, T - 1], mybir.dt.float32) as scan,
        nc.sbuf_tensor([P, W], mybir.dt.float32) as score,
        nc.sbuf_tensor([P, 16], mybir.dt.float32) as opk,
        nc.sbuf_tensor([16, 8], mybir.dt.int16) as idxs,
        nc.semaphore() as dma_in_sem,
        nc.semaphore() as vs,
        nc.semaphore() as v_done,
        nc.semaphore() as prep_sem,
        nc.semaphore() as dma_out_sem,
        nc.Block() as block,
    ):
        pstep = pack[:].ap[0][0]  # partition pitch (int16 elems)

        @block.sync
        def _(sync):
            sync.dma_start(pack[:], pack_in[:]).then_inc(dma_in_sem, 16)

        @block.gpsimd
        def _(gp):
            # Identity row indices for the output scatter, in the SWDGE
            # wrapped-by-16-partitions layout (index j at [j % 16, j // 16]).
            nc.gpsimd.iota(out=idxs[:], pattern=[[16, 8]], base=0,
                           channel_multiplier=1)
            nc.gpsimd.drain()
            # Prepare the output-DMA descriptors DURING the vector compute
            # (descriptors carry addresses only; data is read at trigger
            # time), then fire them the moment v_done lands. This keeps the
            # HWDGE descriptor-generation and DGE->DMA latencies off the
            # critical path.
            ostep = opk[:].ap[0][0]
            in_3d = bass.AP(opk, 0, [[ostep, P], [16, 1], [1, 16]])
            nc.gpsimd.dma_scatter_add(
                out_ap=out_pk[:, 0:16],
                in_ap=in_3d,
                idxs_ap=idxs[:],
                num_idxs=128,
                num_idxs_reg=128,
                elem_size=16,
                elem_step=64,
                prepare_only=True,
                sem=dma_out_sem,
            ).then_inc(prep_sem, 1)
            gp.wait_ge(prep_sem, 1)
            nc.gpsimd.trigger_dma(count=1)._wait_ge(v_done, 1)
            gp.wait_ge(dma_out_sem, 16)

        @block.vector
        def _(vector):
            # NOTE: back-to-back VectorE ops have NO hardware interlock in
            # raw bass — every dependent pair needs an explicit drain()
            # (HW-verified: unfenced chains read stale data).
            drain = nc.vector.drain

            mult = mybir.AluOpType.mult
            add = mybir.AluOpType.add
            is_eq = mybir.AluOpType.is_equal

            # eq[j] = (ranks[j+1] == ranks[j]), j = 0..T-2  (2x: all 2-byte)
            d1 = bass.AP(pack, 1, [[pstep, P], [1, T - 1]])
            d0 = bass.AP(pack, 0, [[pstep, P], [1, T - 1]])
            tt1 = nc.vector.tensor_tensor(out=eq[:], in0=d1, in1=d0, op=is_eq)
            tt1._wait_ge(dma_in_sem, 16)
            drain()
            # segmented scan of contributions: runs reset where eq = 0
            # (contrib AP: contiguous int16 run upcast to the f32 view)
            cshift = bass.AP(
                pack, 2 * (c0 + 1), [[pstep, P], [1, 2 * (T - 1)]]
            ).bitcast(mybir.dt.float32)
            nc.vector.tensor_tensor_scan(out=scan[:], data0=eq[:],
                                         data1=cshift, initial=0.0,
                                         op0=mult, op1=add)
            drain()
            # scored t in [R, R+W): score = scan[t] + eq[t] * NEG_INF
            # (scan out index j = t-1; eq[j=t] = successor-equality, which
            # suppresses every entry that is not its run's last)
            nc.vector.scalar_tensor_tensor(out=score[:], in0=eq[:, R:R + W],
                                           scalar=NEG_INF,
                                           in1=scan[:, R - 1:R - 1 + W],
                                           op0=mult, op1=add)
            drain()
            # per-partition top-8 (values then indices)
            m1 = opk[:, 0:8]
            i1 = opk[:, 8:16].bitcast(mybir.dt.uint32)
            # max -> max_index needs a full semaphore sync (drain is not
            # enough for the 8-wide in_max operand; HW-verified)
            nc.vector.max(out=m1, in_=score[:]).then_inc(vs, 1)
            mi = nc.vector.max_index(out=i1, in_max=m1, in_values=score[:])
            mi._wait_ge(vs, 1)
            mi.then_inc(v_done, 1)

    _slim_program(nc)
    return nc


_BASS_CACHE: dict[tuple[int, int, int], "bass.Bass"] = {}


def _get_bass(T: int, W: int, R: int):
    key = (T, W, R)
    if key not in _BASS_CACHE:
        _BASS_CACHE[key] = _build_bass(T, W, R)
    return _BASS_CACHE[key]


def _gather_entries(ccol, rindices, cvalues, indices, values):
    """Replicate the reference's posting-list gather semantics on host.

    Returns (docs, contrib) 1-D arrays of the valid (unmasked) entries,
    with contrib = cvalues * query weight already applied.
    """
    nnz = rindices.shape[0]
    n_terms = ccol.shape[0] - 1
    L = nnz // n_terms
    idx = indices.reshape(-1).astype(np.int64)
    w = values.reshape(-1).astype(np.float32)
    ccol64 = ccol.astype(np.int64)
    starts = ccol64[idx]
    lens = ccol64[idx + 1] - starts
    eff = np.clip(lens, 0, L)
    offs = np.arange(L, dtype=np.int64)
    mask = offs[None, :] < eff[:, None]
    pos = np.where(mask, starts[:, None] + offs[None, :], 0)
    pos = np.clip(pos, 0, nnz - 1)  # jax gather clamps OOB indices
    docs = rindices[pos]
    contrib = cvalues[pos] * w[:, None]
    m = mask.reshape(-1)
    return (
        docs.reshape(-1)[m].astype(np.int64),
        contrib.reshape(-1)[m].astype(np.float32),
    )


def _host_fallback(docs, contrib, n_docs, top_k):
    """Exact numpy replication of the reference for pathological inputs."""
    acc = np.zeros(n_docs, np.float32)
    ib = (docs >= 0) & (docs < n_docs)  # jax scatter drops OOB updates
    np.add.at(acc, docs[ib], contrib[ib])
    order = np.argsort(-acc, kind="stable")[:top_k]
    return acc[order].astype(np.float32), order.astype(np.int32)


def _first_missing(excluded, count, n_docs):
    """Smallest `count` ids in [0, n_docs) not present in `excluded`."""
    out = []
    excluded = set(int(x) for x in excluded)
    d = 0
    while len(out) < count and d < n_docs:
        if d not in excluded:
            out.append(d)
        d += 1
    return out


def kernel(ccol, rindices, cvalues, indices, values, n_docs, top_k):
    ccol = np.asarray(ccol)
    rindices = np.asarray(rindices)
    cvalues = np.asarray(cvalues)
    indices = np.asarray(indices)
    values = np.asarray(values)
    n_docs = int(n_docs)
    top_k = int(top_k)

    docs, contrib = _gather_entries(ccol, rindices, cvalues, indices, values)
    E = docs.shape[0]

    if E == 0 or top_k > n_docs:
        return _host_fallback(docs, contrib, n_docs, top_k)

    # ---- shard by doc id (sort groups ranges and makes duplicates adjacent)
    order = np.argsort(docs, kind="stable")
    docs_s = docs[order]
    contrib_s = contrib[order]

    # max run of equal doc ids (device unroll depth)
    boundaries = np.flatnonzero(np.diff(docs_s) != 0)
    edges = np.concatenate(([-1], boundaries, [E - 1]))
    max_run = int(np.max(np.diff(edges)))
    if max_run > HALO:
        return _host_fallback(docs, contrib, n_docs, top_k)

    S = -(-n_docs // N_CORES)  # per-core doc range size
    cuts = np.searchsorted(docs_s, np.arange(0, N_CORES + 1) * S)
    shard_lens = np.diff(cuts)
    max_len = int(shard_lens.max())

    W = max(16, -(-max_len // P))
    W = (W + 7) // 8 * 8
    R = max(1, max_run)  # pre-halo depth = max run length
    T = W + R + 2  # R pre-halo + W scored + successor doc col + margin
    FL = (P - 1) * W + T  # flat length backing the P overlapping windows

    if T >= (1 << 15):  # window ranks must fit int16
        return _host_fallback(docs, contrib, n_docs, top_k)

    # ---- build per-core packed int16 [P, 2*XF] tiles (overlapping windows)
    c0 = _c0_f32(T)
    XF = _xf(T)
    win = np.arange(T)[None, :] + (np.arange(P) * W)[:, None]  # [P, T]
    in_maps = []
    shard_docs = []
    for c in range(N_CORES):
        lo, hi = int(cuts[c]), int(cuts[c + 1])
        ln = hi - lo
        # distinct sentinels for unfilled slots keep their ranks distinct
        fdocs = -np.arange(2, FL + 2, dtype=np.int64)
        fcon = np.zeros(FL, np.float32)
        fdocs[R:R + ln] = docs_s[lo:hi]
        fcon[R:R + ln] = contrib_s[lo:hi]
        dw = fdocs[win]  # [P, T] per-window doc ids
        ranks = np.zeros((P, T), np.int16)
        ranks[:, 1:] = np.cumsum(dw[:, 1:] != dw[:, :-1], axis=1)
        arr = np.zeros((P, 2 * XF), np.int16)
        arr[:, 0:T] = ranks
        arr.view(np.float32)[:, c0:c0 + T] = fcon[win]
        in_maps.append({"pack": arr})
        shard_docs.append(docs_s[lo:hi])

    # ---- run on the 8 NeuronCores (retry once on transient NRT errors)
    try:
        nc = _get_bass(T, W, R)
    except Exception as e:  # e.g. bass drift breaking _slim_program's asserts
        import sys
        print(f"kernel: device program build failed ({e!r}); "
              f"falling back to host", file=sys.stderr)
        return _host_fallback(docs, contrib, n_docs, top_k)
    res = None
    last_err = None
    for _attempt in range(2):
        try:
            res = run_bass_kernel_spmd(nc, in_maps,
                                       core_ids=list(range(N_CORES)))
            break
        except Exception as e:  # e.g. transient NRT_EXEC_UNIT_UNRECOVERABLE
            last_err = e
    if res is None:
        import sys
        print(f"kernel: device run failed twice ({last_err!r}); "
              f"falling back to host", file=sys.stderr)
        return _host_fallback(docs, contrib, n_docs, top_k)

    # ---- host reduction of the 8 partial top-8 lists
    cand_docs = []
    cand_scores = []
    part_floor = []  # per-partition 8th-best reported value (exactness check)
    for c in range(N_CORES):
        ln = int(shard_lens[c])
        opk = res.results[c]["out"].reshape(P, 64)[:, 0:16]
        ovals = opk[:, 0:8]
        oidx = opk[:, 8:16].view(np.uint32).astype(np.int64)
        slots = (np.arange(P) * W)[:, None] + oidx  # flat shard position
        valid = (oidx < W) & (slots < ln) & (ovals > -1.0e38)
        # A partition can hide entries only if all 8 of its slots are real
        # (valid) — its unreported entries are <= its 8th-best value.
        full = valid.all(axis=1)
        part_floor.append(np.where(full, ovals[:, 7], -np.inf))
        if valid.any():
            sl = slots[valid]
            cand_docs.append(shard_docs[c][sl].astype(np.int64))
            cand_scores.append(ovals[valid].astype(np.float32))
    part_floor = np.concatenate(part_floor)
    if cand_docs:
        cd = np.concatenate(cand_docs)
        cs = np.concatenate(cand_scores)
    else:
        cd = np.zeros(0, np.int64)
        cs = np.zeros(0, np.float32)

    # defensive dedup by doc id (keep best-ranked entry per doc)
    sel = np.lexsort((cd, -cs))
    cd, cs = cd[sel], cs[sel]
    if len(cd):
        _, first_pos = np.unique(cd, return_index=True)
        keep = np.zeros(len(cd), bool)
        keep[first_pos] = True
        cd, cs = cd[keep], cs[keep]

    # exact top-k of the implicit full score vector (untouched docs score 0),
    # ties broken by lowest doc id (jax.lax.top_k semantics)
    out_vals: list[float] = []
    out_idx: list[int] = []
    i = 0
    while i < len(cs) and len(out_vals) < top_k and cs[i] > 0.0:
        out_vals.append(float(cs[i]))
        out_idx.append(int(cd[i]))
        i += 1
    if len(out_vals) < top_k:
        # zero tier: zero-score candidates and untouched docs, by doc id
        need = top_k - len(out_vals)
        zero_cand = cd[(cs == 0.0)]
        touched = np.unique(docs)
        nonzero_touched = np.setdiff1d(touched, zero_cand, assume_unique=False)
        zero_ids = _first_missing(nonzero_touched, need, n_docs)
        for d in zero_ids[:need]:
            out_vals.append(0.0)
            out_idx.append(int(d))
        # negative tier
        while i < len(cs) and len(out_vals) < top_k:
            if cs[i] < 0.0:
                out_vals.append(float(cs[i]))
                out_idx.append(int(cd[i]))
            i += 1

    # ---- exactness proof for the top-8-per-partition truncation: a
    # partition with 8 valid slots may conceal entries up to its 8th-best
    # value; if that bound reaches the selected k-th value the truncation is
    # not provably exact — recompute on host (astronomically rare). vk < 0
    # is also unprovable (sentinel-zero slots can displace negative leaders
    # without tripping the bound), so it falls back too.
    vk = out_vals[-1] if len(out_vals) == top_k else -np.inf
    if vk < 0.0 or np.any(part_floor >= vk):
        return _host_fallback(docs, contrib, n_docs, top_k)

    return (
        np.asarray(out_vals, np.float32),
        np.asarray(out_idx, np.int32),
    )
